# revision 6
# baseline (speedup 1.0000x reference)
"""Trainium2 Bass kernel for the HSL pairwise-probability layer.

Computation (reference semantics):
    eX[m]   = scatter-mean of X[V[j]] over j with E[j] == m        [M, D]
    hx      = X @ W1[:D];  he = eX @ W1[D:]                        [N, H], [M, H]
    logits  = relu(hx[n] + he[m] + b1) @ W2 + b2                   [N, M]
    out     = sigmoid(logits)

Strategy (8 cores, shard over N):
  - Host: build dense count matrix P[n, m] = #{j : V[j]=n, E[j]=m} (uint8 ->
    fp16, exact), repack b1/W2 into [128, 2] column layout, build +-1
    diagonal sign masks for W2, slice/transpose per-core X slabs. All float
    arithmetic happens on device.
  - Device pre-phase: sumsT[d, m] = sum_n X[n, d] P[n, m] via PE (fp16, counts
    exact); counts via all-ones lhsT; rcp = 1/max(counts, 1) (ACT); eXT =
    sumsT * rcp; heT = W1b^T eXT (PE); pre-scale by |w2| so the heavy
    contraction can use exact +-1 masks; hx path in f32 -> per-partition bias
    table shxb[k, n] = |w2[k]| * (hx[n, k] + b1[k]).
  - Main loop (per output row n): R'[k, m] = relu(sheT[k, m] + shxb[k, n])
    produced in fp16 by ACT (bias trick) + DVE (fused add+max tensor_scalar);
    contracted over k by PE with [128, 32] +-1 diagonal masks into psum row n
    (32x32 PE column tiling, 4 bands concurrent). Tail: sigmoid(logits + b2).
"""

import os

import numpy as np

N, M, D, NNZ, H = 1024, 1024, 128, 32768, 256
NCORES = 8
NS = N // NCORES  # 128 rows of X per core
HC = H // 128  # 2 contraction chunks of 128
ACT_COLS = int(os.environ.get("K_ACT_COLS", "320"))  # ACT/DVE split of chunk0
MAIN_F32 = os.environ.get("K_MAIN_F32", "0") == "1"  # fallback: f32 contraction
COL_TILE = os.environ.get("K_COL_TILE", "1") == "1"  # 32-wide PE column tiling

_LAST = {"exec_time_ns": None, "profile": None}


def _patch_single_wait(tile_mod, mybir):
    """This walrus build rejects >1 sync-wait command per instruction. Replace
    the TileContext tail drain with nop-carried waits (1 wait per nop)."""
    from concourse.vector_clock import ScopedClock

    def _drain_and_barrier(self, tick_clock, wait_clock):
        nop_inst = self.nc.sync.nop(nofuse=True, hint="tail_waits")
        wait_clock.add_sem_waits(
            nop_inst.ins, ScopedClock({None: tick_clock.global_clock})
        )
        si = nop_inst.ins.sync_info
        waits = list(si.on_wait or []) if si else []
        if len(waits) > 1:
            si.on_wait = waits[:1]
            for w in waits[1:]:
                extra = self.nc.sync.nop(nofuse=True, hint="tail_waits2")
                esi = extra.ins.sync_info
                if esi is None:
                    extra.ins.sync_info = mybir.SyncInfo(
                        on_wait=[w], on_update=[]
                    )
                else:
                    esi.on_wait = [w]
        self.nc.sync.drain()
        self.nc.all_engine_barrier()
        popped = self.nc._tile_sem_poison_stack.pop()
        assert popped is self._sem_poison
        self.nc.clear_and_free_semaphores(list(self.sems.allocated().values()))
        self.nc.all_engine_barrier()

    tile_mod.TileContext._drain_and_barrier = _drain_and_barrier


def _split_multi_waits(nc, mybir):
    """Post-pass: split instructions carrying >1 sem wait into nop(wait) +
    instruction(1 wait). Same-engine sequential waits are equivalent."""
    n_split = 0
    for bb in nc.main_func.blocks:
        out = []
        for inst in bb.instructions:
            si = inst.sync_info
            if si is not None and si.on_wait and len(si.on_wait) > 1:
                waits = list(si.on_wait)
                for w in waits[:-1]:
                    nop = mybir.InstNoOp(
                        name=nc.get_next_instruction_name(),
                        opcode="NoOp",
                        engine=inst.engine,
                        ins=[],
                        outs=[],
                        sync_info=mybir.SyncInfo(on_wait=[w], on_update=[]),
                        bass_nofuse=True,
                    )
                    nc.register_instruction(nop, overwrite=True)
                    out.append(nop)
                    n_split += 1
                si.on_wait = [waits[-1]]
            out.append(inst)
        bb.instructions = out
    return n_split


def _build_program(b2_value: float):
    import concourse.bass as bass
    import concourse.mybir as mybir
    import concourse.tile as tile

    _patch_single_wait(tile, mybir)

    f32 = mybir.dt.float32
    f16 = mybir.dt.float16
    Alu = mybir.AluOpType
    Act = mybir.ActivationFunctionType
    CT = f32 if MAIN_F32 else f16

    nc = bass.Bass(num_devices=NCORES)

    xf = nc.dram_tensor("xf", [N, D], f32, kind="ExternalInput")
    p16 = nc.dram_tensor("p16", [N, M], f16, kind="ExternalInput")
    xt = nc.dram_tensor("xt", [D, NS], f32, kind="ExternalInput")
    w1a = nc.dram_tensor("w1a", [D, H], f32, kind="ExternalInput")
    w1b = nc.dram_tensor("w1b", [D, H], f32, kind="ExternalInput")
    b1t = nc.dram_tensor("b1t", [128, HC], f32, kind="ExternalInput")
    w2t = nc.dram_tensor("w2t", [128, HC], f32, kind="ExternalInput")
    masks = nc.dram_tensor("masks", [128, HC * 1024], CT, kind="ExternalInput")
    out_d = nc.dram_tensor("out", [NS, M], f32, kind="ExternalOutput")

    with tile.TileContext(nc) as tc:
        with (
            tc.tile_pool(name="const", bufs=1) as const,
            tc.tile_pool(name="stream", bufs=3) as stream,
            tc.tile_pool(name="r0p", bufs=8) as r0p,
            tc.tile_pool(name="r1p", bufs=8) as r1p,
        ):
            # ---------------- constants in ----------------
            w1a_sb = const.tile([D, H], f32)
            nc.sync.dma_start(out=w1a_sb[:], in_=w1a[:])
            w1b_sb = const.tile([D, H], f32)
            nc.sync.dma_start(out=w1b_sb[:], in_=w1b[:])
            b1t_sb = const.tile([128, HC], f32)
            nc.sync.dma_start(out=b1t_sb[:], in_=b1t[:])
            w2t_sb = const.tile([128, HC], f32)
            nc.sync.dma_start(out=w2t_sb[:], in_=w2t[:])
            masks_sb = const.tile([128, HC * 1024], CT)
            nc.sync.dma_start(out=masks_sb[:], in_=masks[:])
            xt_sb = const.tile([D, NS], f32)
            nc.sync.dma_start(out=xt_sb[:], in_=xt[:])

            w1b16 = const.tile([D, H], f16)
            nc.vector.tensor_copy(out=w1b16[:], in_=w1b_sb[:])
            negw2 = const.tile([128, HC], f32)
            nc.vector.tensor_scalar(
                out=negw2[:], in0=w2t_sb[:], scalar1=-1.0, scalar2=None,
                op0=Alu.mult,
            )
            absw2 = const.tile([128, HC], f32)
            nc.vector.tensor_tensor(
                out=absw2[:], in0=w2t_sb[:], in1=negw2[:], op=Alu.max
            )
            ones16 = const.tile([128, 128], f16)
            nc.vector.memset(ones16[:], 1.0)
            b2col = const.tile([128, 1], f32)
            nc.vector.memset(b2col[:], float(b2_value))

            ext16 = const.tile([128, M], f16)
            rcp_sb = const.tile([128, M], f32)
            she = [const.tile([128, M], f16, tag=f"she{c}", name=f"she{c}") for c in range(HC)]
            shxb = [
                const.tile([128, NS], f32, tag=f"shxb{c}", name=f"shxb{c}")
                for c in range(HC)
            ]
            out_sb = const.tile([128, M], f32)

            # ---------------- stage 1: scatter-sum + counts ----------------
            with tc.tile_pool(name="ps1", bufs=1, space="PSUM") as ps1:
                s_ps = [
                    ps1.tile([128, 512], f32, tag=f"s{h}", name=f"s_ps{h}")
                    for h in range(2)
                ]
                c_ps = [
                    ps1.tile([128, 512], f32, tag=f"c{h}", name=f"c_ps{h}")
                    for h in range(2)
                ]
                for c in range(8):
                    pchunk = stream.tile([128, M], f16, tag="pchunk")
                    nc.sync.dma_start(
                        out=pchunk[:], in_=p16[c * 128 : (c + 1) * 128, :]
                    )
                    xchunk = stream.tile([128, D], f32, tag="xchunk")
                    nc.sync.dma_start(
                        out=xchunk[:], in_=xf[c * 128 : (c + 1) * 128, :]
                    )
                    x16c = stream.tile([128, D], f16, tag="x16c")
                    nc.vector.tensor_copy(out=x16c[:], in_=xchunk[:])
                    for h in range(2):
                        nc.tensor.matmul(
                            s_ps[h][:],
                            lhsT=x16c[:],
                            rhs=pchunk[:, h * 512 : (h + 1) * 512],
                            start=(c == 0),
                            stop=(c == 7),
                        )
                        nc.tensor.matmul(
                            c_ps[h][:],
                            lhsT=ones16[:],
                            rhs=pchunk[:, h * 512 : (h + 1) * 512],
                            start=(c == 0),
                            stop=(c == 7),
                        )
                # rcp = 1 / max(counts, 1)  (counts replicated on partitions)
                for h in range(2):
                    cntf = stream.tile([128, 512], f32, tag="cntf")
                    nc.vector.tensor_scalar(
                        out=cntf[:], in0=c_ps[h][:], scalar1=1.0,
                        scalar2=None, op0=Alu.max,
                    )
                    nc.vector.reciprocal(
                        out=rcp_sb[:, h * 512 : (h + 1) * 512], in_=cntf[:]
                    )
                    nc.vector.tensor_tensor(
                        out=ext16[:, h * 512 : (h + 1) * 512],
                        in0=s_ps[h][:],
                        in1=rcp_sb[:, h * 512 : (h + 1) * 512],
                        op=Alu.mult,
                    )

            # ---------------- stage 2: heT, hx -> bias tables ----------------
            with tc.tile_pool(name="ps2", bufs=2, space="PSUM") as ps2:
                for c in range(HC):
                    for h in range(2):
                        heraw = ps2.tile([128, 512], f32, tag="heraw")
                        nc.tensor.matmul(
                            heraw[:],
                            lhsT=w1b16[:, c * 128 : (c + 1) * 128],
                            rhs=ext16[:, h * 512 : (h + 1) * 512],
                            start=True,
                            stop=True,
                        )
                        nc.vector.tensor_scalar(
                            out=she[c][:, h * 512 : (h + 1) * 512],
                            in0=heraw[:],
                            scalar1=absw2[:, c : c + 1],
                            scalar2=None,
                            op0=Alu.mult,
                        )
                for c in range(HC):
                    hx_ps = ps2.tile([128, NS], f32, tag="hx")
                    nc.tensor.matmul(
                        hx_ps[:],
                        lhsT=w1a_sb[:, c * 128 : (c + 1) * 128],
                        rhs=xt_sb[:],
                        start=True,
                        stop=True,
                    )
                    nc.vector.tensor_scalar(
                        out=shxb[c][:],
                        in0=hx_ps[:],
                        scalar1=b1t_sb[:, c : c + 1],
                        scalar2=absw2[:, c : c + 1],
                        op0=Alu.add,
                        op1=Alu.mult,
                    )

            # ---------------- main loop ----------------
            with tc.tile_pool(name="psm", bufs=1, space="PSUM") as psm:
                lps = [
                    psm.tile([128, 512], f32, tag=f"L{h}", name=f"lps{h}")
                    for h in range(2)
                ]
                n_of = lambda r, j: 32 * j + r
                for r in range(32):
                    rt = []
                    for j in range(4):
                        n = n_of(r, j)
                        r0 = r0p.tile([128, M], CT, tag="r0")
                        r1 = r1p.tile([128, M], CT, tag="r1")
                        if ACT_COLS > 0:
                            nc.scalar.activation(
                                out=r0[:, :ACT_COLS],
                                in_=she[0][:, :ACT_COLS],
                                func=Act.Relu,
                                bias=shxb[0][:, n : n + 1],
                                scale=1.0,
                            )
                        if ACT_COLS < M:
                            nc.vector.tensor_scalar(
                                out=r0[:, ACT_COLS:],
                                in0=she[0][:, ACT_COLS:],
                                scalar1=shxb[0][:, n : n + 1],
                                scalar2=0.0,
                                op0=Alu.add,
                                op1=Alu.max,
                            )
                        nc.vector.tensor_scalar(
                            out=r1[:],
                            in0=she[1][:],
                            scalar1=shxb[1][:, n : n + 1],
                            scalar2=0.0,
                            op0=Alu.add,
                            op1=Alu.max,
                        )
                        rt.append((r0, r1))
                    for c in range(HC):
                        for h in range(2):
                            for j in range(4):
                                rc = rt[j][c]
                                kw = {}
                                if COL_TILE:
                                    kw["tile_position"] = (0, 32 * j)
                                nc.tensor.matmul(
                                    lps[h][32 * j : 32 * j + 32, :],
                                    lhsT=masks_sb[
                                        :,
                                        c * 1024
                                        + r * 32 : c * 1024
                                        + r * 32
                                        + 32,
                                    ],
                                    rhs=rc[:, h * 512 : (h + 1) * 512],
                                    start=(r == 0 and c == 0),
                                    stop=(r == 31 and c == HC - 1),
                                    skip_group_check=True,
                                    **kw,
                                )
                # ---------------- tail: sigmoid + store ----------------
                for h in range(2):
                    nc.scalar.activation(
                        out=out_sb[:, h * 512 : (h + 1) * 512],
                        in_=lps[h][:],
                        func=Act.Sigmoid,
                        bias=b2col[:],
                        scale=1.0,
                    )
            nc.sync.dma_start(out=out_d[:], in_=out_sb[:])

    import concourse.mybir as mybir2

    _split_multi_waits(nc, mybir2)
    return nc


def _host_prep(X, V, E, W1, b1, W2, b2):
    X = np.ascontiguousarray(np.asarray(X, dtype=np.float32))
    V = np.asarray(V).astype(np.int64)
    E = np.asarray(E).astype(np.int64)
    W1 = np.ascontiguousarray(np.asarray(W1, dtype=np.float32))
    b1 = np.asarray(b1, dtype=np.float32)
    W2 = np.asarray(W2, dtype=np.float32)
    b2 = np.asarray(b2, dtype=np.float32)

    cnt = np.zeros((N, M), np.uint16)
    np.add.at(cnt, (V, E), 1)
    p16 = cnt.astype(np.float16)  # exact (counts are tiny integers)

    ct_np = np.float32 if MAIN_F32 else np.float16

    b1t = np.ascontiguousarray(b1.reshape(HC, 128).T)
    w2c = W2[:, 0]
    w2t = np.ascontiguousarray(w2c.reshape(HC, 128).T)
    sgn = np.sign(w2t).astype(ct_np)  # [128, HC]
    masks = np.zeros((128, HC, 32, 32), ct_np)
    for c in range(HC):
        for r in range(32):
            masks[:, c, r, r] = sgn[:, c]
    masks = np.ascontiguousarray(masks.reshape(128, HC * 1024))

    common = {
        "xf": X,
        "p16": p16,
        "w1a": np.ascontiguousarray(W1[:D]),
        "w1b": np.ascontiguousarray(W1[D:]),
        "b1t": b1t,
        "w2t": w2t,
        "masks": masks,
    }
    in_maps = []
    for i in range(NCORES):
        m = dict(common)
        m["xt"] = np.ascontiguousarray(X[i * NS : (i + 1) * NS, :].T)
        in_maps.append(m)
    return in_maps, float(b2.reshape(-1)[0])


def _install_ntff_hook():
    """This image lacks antenv.axon_hooks; synthesize it so
    run_bass_kernel_spmd(trace=True) can capture NTFF profiles via the
    ctypes hook from trn_agent_boot. Also stub the fish artifact upload."""
    import sys
    import types

    try:
        from antenv.axon_hooks import get_axon_ntff_profile_hook  # noqa: F401

        return True
    except ImportError:
        pass
    try:
        import antenv
        from trn_agent_boot.trn_boot import _ntff_profile_via_ctypes
    except ImportError:
        return False
    mod = types.ModuleType("antenv.axon_hooks")
    slot = {"hook": None}
    mod.set_axon_ntff_profile_hook = lambda h: slot.__setitem__("hook", h)
    mod.get_axon_ntff_profile_hook = lambda: slot["hook"]
    sys.modules["antenv.axon_hooks"] = mod
    antenv.axon_hooks = mod
    hook = _ntff_profile_via_ctypes("/opt/axon/libaxon_pjrt.so")
    if hook is None:
        return False
    mod.set_axon_ntff_profile_hook(hook)
    from concourse import bass_utils

    bass_utils.upload_artifacts = lambda tmpdir: str(tmpdir)
    return True


def kernel(X, V, E, W1, b1, W2, b2, _trace=False, _sim=False):
    from concourse.bass_utils import run_bass_kernel_spmd

    in_maps, b2v = _host_prep(X, V, E, W1, b1, W2, b2)
    nc = _build_program(b2v)

    if _sim:
        from concourse import bass_interp

        sim = bass_interp.CoreSim(nc)
        for k, v in in_maps[0].items():
            sim.tensor(k)[:] = v
        sim.simulate()
        out0 = np.array(sim.tensor("out"))
        full = np.concatenate(
            [out0] + [np.zeros((NS, M), np.float32)] * (NCORES - 1), axis=0
        )
        return full

    core_ids = list(range(NCORES))
    if _trace:
        _trace = _install_ntff_hook()
    import tempfile

    tmpdir = tempfile.mkdtemp(prefix="bassk_") if _trace else None
    res = run_bass_kernel_spmd(
        nc, in_maps, core_ids, trace=_trace, tmpdir=tmpdir
    )
    _LAST["trace_dir"] = tmpdir
    _LAST["exec_time_ns"] = res.exec_time_ns
    _LAST["profile"] = res.profile_json
    out = np.concatenate(
        [np.asarray(res.results[i]["out"]) for i in core_ids], axis=0
    )
    return out.astype(np.float32)
